# revision 58
# baseline (speedup 1.0000x reference)
"""Bass/Trainium2 kernel for nn_BiDirectionalCrossAttentionLayer.

Sharding: 8 cores = batch(4) x head-group(2). Each core computes, for its
batch b and its 4 heads, the full 4-stream cross-attention + the 256 output
rows (t = hg*256 .. hg*256+255) of every stream. The reference's
"transpose(1,2) ... transpose/reshape" scramble maps output row t to
(head t//64, head-dim t%64) over all sequence positions, so a head-split of
attention is exactly an output-row split of everything after it.

Schedule: stream-software-pipeline. Attention for stream i+1 (ACT-bound:
16.8M exps/core) is emission-interleaved with the Wo/LN1/FFN/LN2 work of
stream i (PE-bound), so the scalar and tensor engines run concurrently.
attn@V uses an [q, d+1] psum layout (ones column of V gives the softmax
denominator in col 64) so no transposes or psum->sbuf copies are needed;
normalize+accumulate runs on the otherwise-idle Pool (gpsimd) engine.
All matmuls in bf16 (fp32 accumulate); residuals/LN in fp32.
"""

import os
import numpy as np
import ml_dtypes

import concourse.bacc as bacc
import concourse.bass as bass
import concourse.tile as tile
from concourse import mybir
from concourse.bass_utils import run_bass_kernel_spmd
from concourse.masks import make_identity

BF16 = ml_dtypes.bfloat16
F8 = ml_dtypes.float8_e4m3
F32 = np.float32

NS, B, S, E, H, HD = 4, 4, 512, 512, 8, 64
SCALE = HD ** -0.5
LN_EPS = 1e-5
P = 128
HG = 2            # head groups == cores per batch
HPC = 2           # head-pairs per core
HC = H // HG      # heads per core = 4
TG = S // HG      # output rows per core per stream = 256
TS = TG // P      # row tiles per core = 2
ET = E // P       # embedding tiles = 4
KT = S // P       # key/seq tiles = 4
FT = 4 * E // P   # ffn hidden tiles = 16
N_CORES = B * HG

AF = mybir.ActivationFunctionType
ALU = mybir.AluOpType
AX = mybir.AxisListType
DT_BF = mybir.dt.bfloat16
DT_F32 = mybir.dt.float32
DT_F8 = mybir.dt.float8e4
DR = mybir.MatmulPerfMode.DoubleRow


def _build_program(reps=1, ln_triv=True, bf1_zero=True, bf2_zero=True):
    nc = bacc.Bacc("TRN2", target_bir_lowering=False, debug=False)

    def din(name, shape, dt=DT_BF):
        return nc.dram_tensor(name, list(shape), dt, kind="ExternalInput").ap()

    xT_d = din("xT", (NS, P, ET, S), DT_F8)     # xT[n,p,et,s] = x[n,b,s,et*128+p]
    x32_d = din("x32", (NS, P, TS, E), DT_F32)  # x rows t-slice (+bo)
    wq_d = din("wq", (NS, P, ET, HC * HD), DT_F8)  # Wq[n, e, hg*256+c] permuted
    wk_d = din("wk", (NS, P, ET, HC * HD), DT_F8)
    wv_d = din("wv", (NS, P, ET, HC * HD), DT_F8)
    wo_d = din("wo", (NS, P, ET, E))            # Wo[n]/NS, rows e
    w1_d = din("w1", (NS, P, ET, 4 * E))
    w2_d = din("w2", (NS, P, FT, E))
    cmat_d = din("cmat", (P, NS * NS), DT_F32)  # SCALE*inter broadcast on p
    if not ln_triv:
        g1_d = din("g1", (NS, E), DT_F32)
        b1_d = din("b1", (NS, E), DT_F32)
        g2_d = din("g2", (NS, E), DT_F32)
        b2_d = din("b2", (NS, E), DT_F32)
    if not bf1_zero:
        bf1_d = din("bf1", (NS, 4 * E))         # bf1 row (K=1 matmul operand)
    if not bf2_zero:
        bf2_d = din("bf2", (NS, E), DT_F32)
    out_d = nc.dram_tensor("out", [NS, P, TS, E], DT_F32, kind="ExternalOutput").ap()

    with tile.TileContext(nc) as tc:
        with tc.tile_pool(name="const", bufs=1) as const:
            identf = const.tile([P, P], DT_F32)
            make_identity(nc, identf[:])
            cmat_sb = const.tile([P, NS * NS], DT_F32)
            nc.sync.dma_start(cmat_sb[:], cmat_d[:])
            eps_sb = const.tile([P, 1], DT_F32)
            nc.gpsimd.memset(eps_sb[:], LN_EPS)
            # long-lived activations
            r1 = const.tile([P, NS, TS, E], DT_F32)
            r1T = const.tile([P, NS, ET, TG], DT_BF)

            if not ln_triv:
                gbp = tc.alloc_tile_pool(name="gbp", bufs=1)
                g1b = gbp.tile([P, NS, E], DT_F32)
                b1b = gbp.tile([P, NS, E], DT_F32)
                g2b = gbp.tile([P, NS, E], DT_F32)
                b2b = gbp.tile([P, NS, E], DT_F32)
            if not bf2_zero:
                bf2p = tc.alloc_tile_pool(name="bf2p", bufs=1)
                bf2b = bf2p.tile([P, NS, E], DT_F32)
            if not bf1_zero:
                bf1p = tc.alloc_tile_pool(name="bf1p", bufs=1)
                bf1r = bf1p.tile([1, NS, 4 * E], DT_BF)
                ones_row = bf1p.tile([1, TG], DT_BF)
                nc.gpsimd.memset(ones_row[:], 1.0)

            import contextlib
            _loop = tc.For_i(0, reps, 1) if reps > 1 else contextlib.nullcontext()
            with _loop:
              # ---- long-lived per-iteration pools -------------------------
              core = tc.alloc_tile_pool(name="core", bufs=1)
              x32 = core.tile([P, NS, TS, E], DT_F32)
              # fp8 attention operands. qT2[h*32+dl, n, half, s] = q_h[half*32+dl, s]
              qT2 = core.tile([P, NS, 2, S], DT_F8)
              kT2 = core.tile([P, NS, 2, S], DT_F8)
              # engine APs can only start at partition 0/32/64, so head 3's
              # rows (96:128) are DMA-relocated to partitions 0:32 here.
              qT3 = core.tile([P, NS, 2, S], DT_F8)
              kT3 = core.tile([P, NS, 2, S], DT_F8)
              # vex2[k, n, ktpair, h, kthalf, d(+1 ones col)]
              vex2 = core.tile([P, NS, 2, HC, 2, HD + 1], DT_F8)
              wos = core.tile([P, NS, ET, E], DT_BF)
              hp_p = tc.alloc_tile_pool(name="hp_p", bufs=2)  # ffn hidden
              att_p = tc.alloc_tile_pool(name="att_p", bufs=2)
              ex_p = tc.alloc_tile_pool(name="ex_p", bufs=3)
              w1_p = tc.alloc_tile_pool(name="w1_p", bufs=2)   # [P, ET, 1024] halves
              sm = tc.alloc_tile_pool(name="sm", bufs=8)       # [P,*,1] scalars
              tmp = tc.alloc_tile_pool(name="tmp", bufs=2)     # [P,E] f32 temps
              osb = tc.alloc_tile_pool(name="osb", bufs=2)     # out staging

              nc.gpsimd.memset(vex2[:, :, :, :, :, HD:HD + 1], 1.0)

              # ---- psum pools --------------------------------------------
              sps_p = tc.alloc_tile_pool(name="sps_p", bufs=2, space="PSUM")
              vps_p = tc.alloc_tile_pool(name="vps_p", bufs=2, space="PSUM")
              mlp_p = tc.alloc_tile_pool(name="mlp_p", bufs=1, space="PSUM")
              mlp = mlp_p.tile([P, 2, E], DT_F32)           # 2 banks: qkv/wo/ffn

              # ---- input DMAs in priority order --------------------------
              p1 = tc.alloc_tile_pool(name="p1", bufs=1)
              xTs = p1.tile([P, NS, ET, S], DT_F8)
              wqs = p1.tile([P, NS, ET, HC * HD], DT_F8)
              wks = p1.tile([P, NS, ET, HC * HD], DT_F8)
              wvs = p1.tile([P, NS, ET, HC * HD], DT_F8)
              for n in range(NS):
                  nc.sync.dma_start(xTs[:, n], xT_d[n])
                  nc.sync.dma_start(wqs[:, n], wq_d[n])
                  nc.sync.dma_start(wks[:, n], wk_d[n])
                  nc.sync.dma_start(wvs[:, n], wv_d[n])
              for n in range(NS):
                  nc.sync.dma_start(wos[:, n], wo_d[n])
                  nc.sync.dma_start(x32[:, n], x32_d[n])
              w1t = {}  # (n, half) -> tile
              HW1 = 2 * E  # half of 4E
              def fetch_w1(n, half):
                  t = w1_p.tile([P, ET, HW1], DT_BF, tag="w1")
                  nc.sync.dma_start(t[:], w1_d[n][:, :, half * HW1:(half + 1) * HW1])
                  w1t[(n, half)] = t
              w2t = {}
              def fetch_w2(n, half):
                  t = w1_p.tile([P, FT // 2, E], DT_BF, tag="w2")
                  nc.sync.dma_start(t[:], w2_d[n][:, half * (FT // 2):(half + 1) * (FT // 2)])
                  w2t[(n, half)] = t
              fetch_w1(0, 0)
              fetch_w1(0, 1)
              fetch_w2(0, 0)
              fetch_w2(0, 1)
              if not bf1_zero:
                  nc.sync.dma_start(bf1r[:], bf1_d[None, :, :])
              if not ln_triv:
                  for n in range(NS):
                      nc.sync.dma_start(g1b[:, n], g1_d[n].partition_broadcast(P))
                      nc.sync.dma_start(b1b[:, n], b1_d[n].partition_broadcast(P))
                      nc.sync.dma_start(g2b[:, n], g2_d[n].partition_broadcast(P))
                      nc.sync.dma_start(b2b[:, n], b2_d[n].partition_broadcast(P))
              if not bf2_zero:
                  for n in range(NS):
                      nc.sync.dma_start(bf2b[:, n], bf2_d[n].partition_broadcast(P))

              # ---- building blocks ---------------------------------------
              def emit_qkv_qk(n, which):
                  # wq/wk are host-permuted so matmul group `half` yields all
                  # four heads' d-rows [half*32, half*32+32) on partitions
                  # h*32+dl — the DoubleRow layout for fp8 scores.
                  ws = wqs if which == "q" else wks
                  dst = qT2 if which == "q" else kT2
                  for half in range(2):
                      for ep in range(ET // 2):
                          nc.tensor.matmul(
                              mlp[:, half, :],
                              ws[:, n, 2 * ep:2 * ep + 2, half * P:(half + 1) * P],
                              xTs[:, n, 2 * ep:2 * ep + 2, :],
                              start=(ep == 0), stop=(ep == ET // 2 - 1),
                              perf_mode=DR)
                  for half in range(2):
                      nc.vector.tensor_copy(dst[:, n, half], mlp[:, half, :])
                  dst3 = qT3 if which == "q" else kT3
                  nc.scalar.dma_start(dst3[0:32, n], dst[96:128, n])

              def emit_qkv_v(n):
                  for kt in range(KT):
                      dstp = mlp[:, kt // 2, (kt % 2) * 256:(kt % 2) * 256 + 256]
                      for ep in range(ET // 2):
                          nc.tensor.matmul(
                              dstp,
                              xTs[:, n, 2 * ep:2 * ep + 2, kt * P:(kt + 1) * P],
                              wvs[:, n, 2 * ep:2 * ep + 2, :],
                              start=(ep == 0), stop=(ep == ET // 2 - 1),
                              perf_mode=DR)
                  for kt in range(KT):
                      srcp = mlp[:, kt // 2, (kt % 2) * 256:(kt % 2) * 256 + 256]
                      nc.vector.tensor_copy(
                          vex2[:, n, kt // 2, :, kt % 2, 0:HD],
                          srcp.rearrange("p (h d) -> p h d", d=HD))

              # pending attention head: (i, j, h, ex, att_t); its attn@V +
              # normalize are emitted one head-slot later so the PE never
              # hits a head-of-line wait on the exp ACTIVATE.
              pend = [None]

              def _v_steps(pv):
                  # Generator of the pending head's 8 attn@V DR matmuls.
                  # Their 256-row stationary loads only hide under long
                  # matmuls, so the caller interleaves them with the next
                  # head's 256-cycle score matmuls.
                  i, j, h, exs, att_t, vt = pv
                  for qt in range(KT):
                      for tp in range(2):
                          nc.tensor.matmul(
                              vt[:, qt, 0:HD + 1],
                              exs[tp][:, :, qt * P:(qt + 1) * P],
                              vex2[:, j, tp, h, :, :], start=(tp == 0),
                              stop=(tp == 1), perf_mode=DR)
                          yield

              def _v_finish(pv):
                  i, j, h, exs, att_t, vt = pv
                  r4 = sm.tile([P, KT, 1], DT_F32, tag="r4", bufs=3)
                  nc.vector.reciprocal(r4[:], vt[:, :, HD:HD + 1])
                  for qt in range(KT):
                      dst = att_t[:, qt, h * HD:(h + 1) * HD]
                      if j == 0:
                          nc.vector.tensor_scalar_mul(dst, vt[:, qt, 0:HD],
                                                      r4[:, qt])
                      else:
                          nc.vector.scalar_tensor_tensor(
                              out=dst, in0=vt[:, qt, 0:HD], scalar=r4[:, qt],
                              in1=dst, op0=ALU.mult, op1=ALU.add)

              def flush_pend():
                  if pend[0] is None:
                      return
                  pv = pend[0]
                  pend[0] = None
                  for _ in _v_steps(pv):
                      pass
                  _v_finish(pv)

              def emit_head(i, j, h, att_t):
                  if h == 3:
                      pr = slice(0, 32)
                      kk, qq = kT3, qT3
                  else:
                      pr = slice(h * 32, (h + 1) * 32)
                      kk, qq = kT2, qT2
                  c_ap = cmat_sb[:, (i * NS + j):(i * NS + j + 1)]
                  pv = pend[0]
                  pend[0] = None
                  vsteps = iter(_v_steps(pv)) if pv is not None else iter(())
                  exs = []
                  for half in range(2):
                      sps = sps_p.tile([P, 2, S], DT_F32, tag="sps", name="sps")
                      for k2 in range(2):
                          kt = half * 2 + k2
                          nc.tensor.matmul(
                              sps[:, k2, :], kk[pr, j, :, kt * P:(kt + 1) * P],
                              qq[pr, i, :, :], start=True, stop=True,
                              perf_mode=DR)
                          next(vsteps, None)
                          next(vsteps, None)
                      ex = ex_p.tile([P, 2, S], DT_F8, tag="ex")
                      nc.scalar.activation(
                          ex[:].rearrange("p a b -> p (a b)"),
                          sps[:].rearrange("p a b -> p (a b)"), AF.Exp, scale=c_ap)
                      exs.append(ex)
                  for _ in vsteps:
                      pass
                  if pv is not None:
                      _v_finish(pv)
                  vt = vps_p.tile([P, KT, P], DT_F32, tag="vps", name="vt")
                  pend[0] = (i, j, h, exs, att_t, vt)

              def ln_stats(src_ap, res_ap, var_sl):
                  # y = src+res; xc = y - mean; var_sl[P,1] = var(y)+eps.
                  # Returns xc. No ACT involvement (avoids table loads).
                  y = tmp.tile([P, E], DT_F32, tag="y")
                  msum = sm.tile([P, 1], DT_F32, tag="msum")
                  nc.vector.scalar_tensor_tensor(
                      out=y[:], in0=src_ap, scalar=1.0, in1=res_ap,
                      op0=ALU.mult, op1=ALU.add, accum_out=msum[:])
                  nm = sm.tile([P, 1], DT_F32, tag="nm")
                  nc.vector.tensor_scalar_mul(nm[:], msum[:], -1.0 / E)
                  xc = tmp.tile([P, E], DT_F32, tag="xc")
                  nc.vector.tensor_scalar_add(xc[:], y[:], nm[:])
                  ssum = sm.tile([P, 1], DT_F32, tag="ssum")
                  nc.vector.scalar_tensor_tensor(
                      out=y[:], in0=xc[:], scalar=1.0, in1=xc[:],
                      op0=ALU.mult, op1=ALU.mult, accum_out=ssum[:])
                  nc.vector.tensor_scalar(
                      out=var_sl, in0=ssum[:], scalar1=1.0 / E, scalar2=LN_EPS,
                      op0=ALU.mult, op1=ALU.add)
                  return xc

              def newton_rsqrt(v, w=TS):
                  # v: [P, w] variance+eps (≈1 for LN'd residual streams).
                  # DVE-only invstd: seed 1.5-0.5v, 4 Newton steps.
                  s = sm.tile([P, w], DT_F32, tag="nr_s", name="nr_s")
                  t = sm.tile([P, w], DT_F32, tag="nr_t", name="nr_t")
                  nc.vector.tensor_scalar(
                      out=s[:], in0=v, scalar1=-0.5, scalar2=1.5,
                      op0=ALU.mult, op1=ALU.add)
                  for _ in range(4):
                      nc.vector.tensor_tensor(t[:], s[:], s[:], ALU.mult)
                      nc.vector.tensor_tensor(t[:], t[:], v, ALU.mult)
                      nc.vector.tensor_scalar(
                          out=t[:], in0=t[:], scalar1=-0.5, scalar2=1.5,
                          op0=ALU.mult, op1=ALU.add)
                      nc.vector.tensor_tensor(s[:], s[:], t[:], ALU.mult)
                  return s

              def ln_out(xc, inv_sl, out_ap, n, which):
                  if ln_triv:
                      nc.vector.tensor_scalar_mul(out_ap, xc[:], inv_sl)
                  else:
                      g = g1b if which == 1 else g2b
                      b = b1b if which == 1 else b2b
                      nc.vector.scalar_tensor_tensor(
                          out=out_ap, in0=xc[:], scalar=inv_sl, in1=g[:, n],
                          op0=ALU.mult, op1=ALU.mult)
                      nc.vector.tensor_add(out_ap, out_ap, b[:, n])

              def make_fillers(i, att_t, drain=False):
                  # Closures for stream i's post-attention work. `head` runs
                  # during stream i+1's pairs; `tail` (gelu+FFN2+LN2, which
                  # gate on all of FFN1) is deferred to the following step so
                  # the gelu never bubbles the ACT queue mid-step. In drain
                  # mode (last stream, nothing left to overlap) the gelu is
                  # split in half and woven between FFN1 chunks, and FFN2
                  # accumulates per gelu-half, to keep the PE warm.
                  hpre = hp_p.tile([P, FT, TG], DT_BF, tag="hpre",
                                   name=f"hpre{i}")

                  var1 = sm.tile([P, TS], DT_F32, tag="var", name=f"var1_{i}")
                  xcs = {}

                  def wo_ts(ts):
                      def f():
                          wo_ps = mlp[:, ts, :]
                          for qt in range(KT):
                              nc.tensor.matmul(
                                  wo_ps, att_t[:, qt, ts * P:(ts + 1) * P],
                                  wos[:, i, qt], start=(qt == 0),
                                  stop=(qt == KT - 1))
                          xcs[ts] = ln_stats(wo_ps, x32[:, i, ts],
                                             var1[:, ts:ts + 1])
                          if ts == TS - 1:
                              inv = newton_rsqrt(var1[:])
                              for t2 in range(TS):
                                  ln_out(xcs[t2], inv[:, t2:t2 + 1],
                                         r1[:, i, t2], i, 1)
                      return f

                  def tr_ts(ts):
                      def f():
                          for et in range(ET):
                              rt = mlp[:, ts, et * P:(et + 1) * P]
                              nc.tensor.matmul(rt, r1[:, i, ts, et * P:(et + 1) * P],
                                               identf[:], is_transpose=True,
                                               start=True, stop=True)
                          for et in range(ET):
                              rt = mlp[:, ts, et * P:(et + 1) * P]
                              nc.vector.tensor_copy(
                                  r1T[:, i, et, ts * P:(ts + 1) * P], rt)
                      return f

                  def f1_c(c):
                      def f():
                          half, w1s = c // 4, w1t[(i, c // 4)]
                          hp2 = mlp[:, c % 2, :]
                          for s2 in range(2):
                              fs_l = (c % 4) * 2 + s2   # column within half
                              dstp = hp2[:, s2 * 256:(s2 + 1) * 256]
                              for et in range(ET):
                                  nc.tensor.matmul(
                                      dstp, w1s[:, et, fs_l * P:(fs_l + 1) * P],
                                      r1T[:, i, et], start=(et == 0),
                                      stop=(et == ET - 1) if bf1_zero else False)
                              if not bf1_zero:
                                  fs = half * 8 + fs_l
                                  nc.tensor.matmul(
                                      dstp, bf1r[0:1, i, fs * P:(fs + 1) * P],
                                      ones_row[:], start=False, stop=True)
                          nc.vector.tensor_copy(
                              hpre[:, 2 * c:2 * c + 2, :].rearrange("p a b -> p (a b)"),
                              hp2)
                          if c == 3 and i < NS - 1:
                              fetch_w1(i + 1, 0)
                          if c == 7 and i < NS - 1:
                              fetch_w1(i + 1, 1)
                          if drain and c in (3, 7):
                              gelu_part(c // 4)
                      return f

                  def gelu_part(g):
                      v = hpre[:, g * (FT // 2):(g + 1) * (FT // 2), :]
                      nc.scalar.activation(v.rearrange("p a b -> p (a b)"),
                                           v.rearrange("p a b -> p (a b)"),
                                           AF.Gelu)

                  def gelu():
                      nc.scalar.activation(
                          hpre[:].rearrange("p a b -> p (a b)"),
                          hpre[:].rearrange("p a b -> p (a b)"), AF.Gelu)

                  out_sb = osb.tile([P, TS, E], DT_F32, tag="osb")
                  var2 = sm.tile([P, TS], DT_F32, tag="var", name=f"var2_{i}")
                  xc2s = {}

                  def f2_ts(ts, fh=None):
                      def f():
                          f2 = mlp[:, ts, :]
                          fts = range(FT) if fh is None else \
                              range(fh * (FT // 2), (fh + 1) * (FT // 2))
                          for ft in fts:
                              w2s = w2t[(i, ft // 8)]
                              nc.tensor.matmul(
                                  f2, hpre[:, ft, ts * P:(ts + 1) * P],
                                  w2s[:, ft % 8], start=(ft == 0),
                                  stop=(ft == FT - 1))
                          if fh == 0:
                              return
                          if bf2_zero:
                              res = r1[:, i, ts]
                          else:
                              res = tmp.tile([P, E], DT_F32, tag="res")
                              nc.vector.tensor_add(res[:], r1[:, i, ts], bf2b[:, i])
                              res = res[:]
                          xc2s[ts] = ln_stats(f2, res, var2[:, ts:ts + 1])
                          if ts == 0 and i < NS - 1:
                              fetch_w2(i + 1, 0)
                          if i == NS - 1:
                              # drain stream: finish each ts independently so
                              # LN2/out of ts0 overlaps FFN2 of ts1
                              inv = newton_rsqrt(var2[:, ts:ts + 1], 1)
                              ln_out(xc2s[ts], inv[:, 0:1], out_sb[:, ts], i, 2)
                              nc.sync.dma_start(out_d[i][:, ts], out_sb[:, ts])
                          elif ts == TS - 1:
                              inv = newton_rsqrt(var2[:])
                              for t2 in range(TS):
                                  ln_out(xc2s[t2], inv[:, t2:t2 + 1],
                                         out_sb[:, t2], i, 2)
                              nc.sync.dma_start(out_d[i], out_sb[:])
                              fetch_w2(i + 1, 1)
                      return f

                  head = [wo_ts(0), wo_ts(1), tr_ts(0), tr_ts(1)]
                  head += [f1_c(c) for c in range(8)]
                  if drain:
                      tail = [f2_ts(0, 0), f2_ts(0, 1), f2_ts(1, 0),
                              f2_ts(1, 1)]
                  else:
                      tail = [gelu, f2_ts(0), f2_ts(1)]
                  return head, tail

              def interleave(tail_prev, head):
                  # gelu(i-1) trails the first exps of the step in the ACT
                  # queue (no step-boundary ACT stall); FFN2(i-1) lands
                  # several slots later so the PE never waits on the gelu.
                  if not tail_prev:
                      return head
                  gl, f2a, f2b = tail_prev
                  return (head[0:2] + [gl] + head[2:6] + [f2a]
                          + head[6:8] + [f2b] + head[8:])

              # ---- emission ----------------------------------------------
              att_tiles = {}

              def get_att(i):
                  if i not in att_tiles:
                      att_tiles[i] = att_p.tile([P, KT, HC * HD], DT_BF,
                                                tag="att", name=f"att{i}")
                  return att_tiles[i]

              def emit_pair(i, j, fillers):
                  att_t = get_att(i)
                  for h in range(HC):
                      if fillers:
                          fillers.pop(0)()
                      emit_head(i, j, h, att_t)

              # prologue: QKV(0), then stream 0's pairs fed by QKV(1..3)
              emit_qkv_qk(0, "q")
              emit_qkv_qk(0, "k")
              emit_qkv_v(0)
              pro = []
              for n in range(1, NS):
                  pro.append(lambda n=n: emit_qkv_qk(n, "q"))
                  pro.append(lambda n=n: emit_qkv_qk(n, "k"))
                  pro.append(lambda n=n: emit_qkv_v(n))
              for j in range(NS):
                  emit_pair(0, j, pro)
              while pro:
                  pro.pop(0)()
              p1.release()

              # steady state: pairs(i+1) interleaved with head-fillers(i) and
              # the deferred tail (gelu/FFN2/LN2) of stream i-1.
              prev_tail = []
              for i in range(NS):
                  flush_pend()  # complete att(i) before its Wo consumes it
                  head, tail = make_fillers(i, get_att(i), drain=(i == NS - 1))
                  fl = interleave(prev_tail, head)
                  prev_tail = tail
                  if i + 1 < NS:
                      for j in range(NS):
                          emit_pair(i + 1, j, fl)
                  for f in fl:
                      f()
              for f in prev_tail:
                  f()

              for p in (mlp_p, vps_p, sps_p, osb, tmp, sm, w1_p, ex_p,
                        att_p, hp_p, core):
                  p.release()

    nc.compile()
    return nc


_NC_CACHE = {}


def _get_nc(reps=1, variant=(True, True, True)):
    key = (reps,) + tuple(variant)
    if key not in _NC_CACHE:
        _NC_CACHE[key] = _build_program(reps, *variant)
    return _NC_CACHE[key]


def _variant(inputs):
    ln_triv = (np.all(np.asarray(inputs["ln1_g"]) == 1.0)
               and np.all(np.asarray(inputs["ln2_g"]) == 1.0)
               and not np.any(np.asarray(inputs["ln1_b"]))
               and not np.any(np.asarray(inputs["ln2_b"])))
    return (ln_triv, not np.any(np.asarray(inputs["bf1"])),
            not np.any(np.asarray(inputs["bf2"])))


def _pack_inputs(x0, x1, x2, x3, Wq, Wk, Wv, Wo, bo, ln1_g, ln1_b, ln2_g, ln2_b,
                 W1, bf1, W2, bf2, inter):
    variant = _variant(dict(ln1_g=ln1_g, ln1_b=ln1_b, ln2_g=ln2_g, ln2_b=ln2_b,
                            bf1=bf1, bf2=bf2))
    ln_triv, bf1_zero, bf2_zero = variant
    x = np.stack([np.asarray(x0), np.asarray(x1), np.asarray(x2),
                  np.asarray(x3)]).astype(F32)  # [NS,B,S,E]
    Wq, Wk, Wv, Wo = (np.asarray(a, F32) for a in (Wq, Wk, Wv, Wo))
    inputs_bo = np.asarray(bo, F32)
    W1, W2 = np.asarray(W1, F32), np.asarray(W2, F32)
    inter = np.asarray(inter, F32)

    def tile_rows(a, nt):
        # [NS, R, C] -> [NS, P, nt, C]
        return np.ascontiguousarray(
            a.reshape(NS, nt, P, a.shape[-1]).transpose(0, 2, 1, 3))

    shared = {
        "wo": tile_rows(Wo / NS, ET).astype(BF16),
        "w1": tile_rows(W1, ET).astype(BF16),
        "w2": tile_rows(W2, FT).astype(BF16),
        "cmat": np.ascontiguousarray(
            np.broadcast_to((inter * SCALE).reshape(1, NS * NS), (P, NS * NS))
        ).astype(F32),
    }
    if not ln_triv:
        shared.update({
            "g1": np.ascontiguousarray(ln1_g, dtype=F32),
            "b1": np.ascontiguousarray(ln1_b, dtype=F32),
            "g2": np.ascontiguousarray(ln2_g, dtype=F32),
            "b2": np.ascontiguousarray(ln2_b, dtype=F32),
        })
    if not bf1_zero:
        shared["bf1"] = np.ascontiguousarray(np.asarray(bf1, F32)).astype(BF16)
    if not bf2_zero:
        shared["bf2"] = np.ascontiguousarray(bf2, dtype=F32)
    # q/k column permutation: matmul group `half` (128 cols) holds all four
    # heads' d-rows [half*32, half*32+32) so the fp8 DoubleRow scores layout
    # qT2[h*32+dl, half, s] falls out of a plain psum->sbuf copy.
    perm = np.array([h * HD + half * 32 + dl
                     for half in range(2) for h in range(HC)
                     for dl in range(32)])
    per_hg = []
    for hg in range(HG):
        cols = slice(hg * HC * HD, (hg + 1) * HC * HD)
        per_hg.append({
            "wq": tile_rows(Wq[:, :, cols][:, :, perm], ET).astype(F8),
            "wk": tile_rows(Wk[:, :, cols][:, :, perm], ET).astype(F8),
            "wv": tile_rows(Wv[:, :, cols], ET).astype(F8),
        })
    xT_b = {}
    for b in range(B):
        xT_b[b] = np.ascontiguousarray(
            x[:, b].transpose(0, 2, 1).reshape(NS, ET, P, S).transpose(0, 2, 1, 3)
        ).astype(F8)
    in_maps = []
    for core in range(N_CORES):
        b, hg = core // HG, core % HG
        xb = x[:, b]  # [NS, S, E]
        x32 = np.ascontiguousarray(
            (xb[:, hg * TG:(hg + 1) * TG] + inputs_bo[:, None, :])
            .reshape(NS, TS, P, E).transpose(0, 2, 1, 3).astype(F32))
        m = {"xT": xT_b[b], "x32": x32}
        m.update(shared)
        m.update(per_hg[hg])
        in_maps.append(m)
    return variant, in_maps


def _unpack_outputs(results):
    full = np.empty((NS, B, S, E), dtype=F32)
    for core in range(N_CORES):
        b, hg = core // HG, core % HG
        o = results[core]["out"]  # [NS, P, TS, E]
        full[:, b, hg * TG:(hg + 1) * TG] = (
            o.transpose(0, 2, 1, 3).reshape(NS, TG, E))
    return tuple(full[n] for n in range(NS))


def _get_exec(variant):
    """Cached jitted shard_map executable for the kernel (mirrors the
    run_bass_kernel_spmd -> run_bass_via_pjrt lowering, but reusable across
    calls instead of re-tracing/jitting per invocation)."""
    key = ("exec",) + tuple(variant)
    if key in _NC_CACHE:
        return _NC_CACHE[key]
    import jax
    from jax.sharding import Mesh, PartitionSpec
    from jax.experimental.shard_map import shard_map
    from concourse import bass2jax
    from concourse import mybir as mb

    nc = _get_nc(1, variant)
    bass2jax.install_neuronx_cc_hook()
    part_name = nc.partition_id_tensor.name if nc.partition_id_tensor else None
    in_names, out_names, out_avals, zero_outs = [], [], [], []
    for alloc in nc.m.functions[0].allocations:
        if not isinstance(alloc, mb.MemoryLocationSet):
            continue
        name = alloc.memorylocations[0].name
        if alloc.kind == "ExternalInput":
            if name != part_name:
                in_names.append(name)
        elif alloc.kind == "ExternalOutput":
            out_names.append(name)
            shape = tuple(alloc.tensor_shape)
            dtype = mb.dt.np(alloc.dtype)
            out_avals.append(jax.core.ShapedArray(shape, dtype))
            zero_outs.append(np.zeros(shape, dtype))
    n_params = len(in_names)
    all_names = in_names + out_names
    if part_name is not None:
        all_names = all_names + [part_name]

    def _body(*args):
        operands = list(args)
        if part_name is not None:
            operands.append(bass2jax.partition_id_tensor())
        outs = bass2jax._bass_exec_p.bind(
            *operands, out_avals=tuple(out_avals), in_names=tuple(all_names),
            out_names=tuple(out_names), lowering_input_output_aliases=(),
            sim_require_finite=True, sim_require_nnan=True, nc=nc)
        return tuple(outs)

    devices = jax.devices()[:N_CORES]
    mesh = Mesh(np.asarray(devices), ("core",))
    spec = PartitionSpec("core")
    donate = tuple(range(n_params, n_params + len(out_names)))
    fn = jax.jit(shard_map(
        _body, mesh=mesh, in_specs=(spec,) * (n_params + len(out_names)),
        out_specs=(spec,) * len(out_names), check_rep=False),
        donate_argnums=donate, keep_unused=True)
    _NC_CACHE[key] = (fn, in_names, out_avals, zero_outs)
    return _NC_CACHE[key]


def kernel(**inputs):
    variant, in_maps = _pack_inputs(**inputs)
    if int(os.environ.get("KERNEL_TRACE", "0")):
        nc = _get_nc(1, variant)
        res = run_bass_kernel_spmd(
            nc, in_maps, core_ids=list(range(N_CORES)), trace=True)
        _NC_CACHE["last_result"] = res
        return _unpack_outputs(res.results)
    fn, in_names, out_avals, zero_outs = _get_exec(variant)
    concat = [np.concatenate([in_maps[c][nm] for c in range(N_CORES)], axis=0)
              for nm in in_names]
    concat += [np.zeros((N_CORES * z.shape[0], *z.shape[1:]), z.dtype)
               for z in zero_outs]
    outs = fn(*concat)
    o = np.asarray(outs[0]).reshape(N_CORES, *out_avals[0].shape)
    return _unpack_outputs([{"out": o[c]} for c in range(N_CORES)])


def bench(inputs, iters=20, reps=1, phases="all"):
    """Time the on-device execution with device-resident inputs.

    Mirrors bass2jax.run_bass_via_pjrt's shard_map(_bass_exec) lowering but
    without output-buffer donation, so the same executable can be re-invoked
    and timed. Returns (min, median) seconds per call. With reps>1 the NEFF
    contains the kernel body replicated; use slopes across reps to cancel
    the fixed axon-RPC dispatch overhead.
    """
    import time
    import jax
    from jax.sharding import Mesh, PartitionSpec, NamedSharding
    from jax.experimental.shard_map import shard_map
    from concourse import bass2jax
    from concourse import mybir as mb

    variant, in_maps = _pack_inputs(**inputs)
    nc = _get_nc(reps, variant)
    bass2jax.install_neuronx_cc_hook()

    part_name = nc.partition_id_tensor.name if nc.partition_id_tensor else None
    in_names, out_names, out_avals, zero_outs = [], [], [], []
    for alloc in nc.m.functions[0].allocations:
        if not isinstance(alloc, mb.MemoryLocationSet):
            continue
        name = alloc.memorylocations[0].name
        if alloc.kind == "ExternalInput":
            if name != part_name:
                in_names.append(name)
        elif alloc.kind == "ExternalOutput":
            out_names.append(name)
            shape = tuple(alloc.tensor_shape)
            dtype = mb.dt.np(alloc.dtype)
            out_avals.append(jax.core.ShapedArray(shape, dtype))
            zero_outs.append(np.zeros(shape, dtype))
    n_params = len(in_names)
    all_names = in_names + out_names
    if part_name is not None:
        all_names = all_names + [part_name]

    def _body(*args):
        operands = list(args)
        if part_name is not None:
            operands.append(bass2jax.partition_id_tensor())
        outs = bass2jax._bass_exec_p.bind(
            *operands, out_avals=tuple(out_avals), in_names=tuple(all_names),
            out_names=tuple(out_names), lowering_input_output_aliases=(),
            sim_require_finite=True, sim_require_nnan=True, nc=nc)
        return tuple(outs)

    devices = jax.devices()[:N_CORES]
    mesh = Mesh(np.asarray(devices), ("core",))
    spec = PartitionSpec("core")
    fn = jax.jit(shard_map(
        _body, mesh=mesh, in_specs=(spec,) * (n_params + len(out_names)),
        out_specs=(spec,) * len(out_names), check_rep=False))
    sh = NamedSharding(mesh, spec)
    concat = [jax.device_put(
        np.concatenate([in_maps[c][nm] for c in range(N_CORES)], axis=0), sh)
        for nm in in_names]
    concat += [jax.device_put(
        np.zeros((N_CORES * z.shape[0], *z.shape[1:]), z.dtype), sh)
        for z in zero_outs]

    out = fn(*concat)  # compile
    jax.block_until_ready(out)
    times = []
    for _ in range(iters):
        t0 = time.perf_counter()
        out = fn(*concat)
        jax.block_until_ready(out)
        times.append(time.perf_counter() - t0)
    times.sort()
    return times[0], times[len(times) // 2]


if __name__ == "__main__":
    import sys
    mode = sys.argv[1] if len(sys.argv) > 1 else "sim"
    sys.path.insert(0, os.path.dirname(os.path.abspath(__file__)))
    import reference

    inputs = {k: np.asarray(v) for k, v in reference.setup_inputs().items()}
    if mode == "sim":
        # Simulate core 0 (b=0, hg=0) with CoreSim and compare to reference.
        # CoreSim has no Gelu; patch exact erf-gelu into its activation visitor.
        import concourse.bass_interp as bass_interp
        from scipy.special import erf as _erf
        _orig_visit = bass_interp.InstructionExecutor.visit_InstActivation

        def _patched(self, instruction, reg_snapshot=None):
            if instruction.func == mybir.ActivationFunctionType.Gelu:
                instruction.func = mybir.ActivationFunctionType.Identity
                try:
                    from concourse.bass_interp import Direction
                    out_ap = instruction.outs[0]
                    res = _orig_visit(self, instruction, reg_snapshot=reg_snapshot)
                    v = self.view_ap(out_ap, Direction.WRITE, instruction,
                                     reg_snapshot=reg_snapshot)
                    x = v[:].astype(np.float32)
                    v[:] = (x * 0.5 * (1.0 + _erf(x / np.sqrt(2.0)))).astype(v.dtype)
                    return res
                finally:
                    instruction.func = mybir.ActivationFunctionType.Gelu
            return _orig_visit(self, instruction, reg_snapshot=reg_snapshot)

        bass_interp.InstructionExecutor.visit_InstActivation = _patched
        from concourse.bass_interp import CoreSim
        core_id = int(sys.argv[2]) if len(sys.argv) > 2 else 0
        variant, in_maps = _pack_inputs(**inputs)
        nc = _get_nc(1, variant)
        sim = CoreSim(nc, trace=False)
        for name, arr in in_maps[core_id].items():
            sim.tensor(name)[:] = arr
        sim.simulate(check_with_hw=False)
        out = sim.tensor("out").copy()
        got = out.transpose(0, 2, 1, 3).reshape(NS, TG, E)
        exp = np.stack([np.asarray(o) for o in reference.reference(**inputs)])
        b, hg = core_id // HG, core_id % HG
        exp_slice = exp[:, b, hg * TG:(hg + 1) * TG]
        err = np.abs(got - exp_slice)
        rel = np.linalg.norm(got - exp_slice) / np.linalg.norm(exp_slice)
        print(f"max abs err: {err.max():.3e}  rel fro err: {rel:.3e}")
    elif mode == "tsim":
        from concourse.timeline_sim import TimelineSim
        variant, in_maps = _pack_inputs(**inputs)
        nc = _get_nc(1, variant)
        ts = TimelineSim(nc, trace=False)
        t = ts.simulate()
        print(f"TimelineSim predicted: {t:.0f} ns")
    else:
        got = kernel(**inputs)
        exp = reference.reference(**inputs)
        for n in range(NS):
            g, e = np.asarray(got[n]), np.asarray(exp[n])
            rel = np.linalg.norm(g - e) / np.linalg.norm(e)
            print(f"out{n}: rel fro err {rel:.3e} max abs {np.abs(g - e).max():.3e}")


# revision 68
# speedup vs baseline: 1.1197x; 1.1197x over previous
"""Bass/Trainium2 kernel for nn_BiDirectionalCrossAttentionLayer.

Sharding: 8 cores = batch(4) x head-group(2). Each core computes, for its
batch b and its 4 heads, the full 4-stream cross-attention + the 256 output
rows (t = hg*256 .. hg*256+255) of every stream. The reference's
"transpose(1,2) ... transpose/reshape" scramble maps output row t to
(head t//64, head-dim t%64) over all sequence positions, so a head-split of
attention is exactly an output-row split of everything after it.

Schedule: stream-software-pipeline. Attention for stream i+1 (ACT-bound:
16.8M exps/core) is emission-interleaved with the Wo/LN1/FFN/LN2 work of
stream i (PE-bound), so the scalar and tensor engines run concurrently.
attn@V uses an [q, d+1] psum layout (ones column of V gives the softmax
denominator in col 64) so no transposes or psum->sbuf copies are needed;
normalize+accumulate runs on the otherwise-idle Pool (gpsimd) engine.
All matmuls in bf16 (fp32 accumulate); residuals/LN in fp32.
"""

import os
import numpy as np
import ml_dtypes

import concourse.bacc as bacc
import concourse.bass as bass
import concourse.tile as tile
from concourse import mybir
from concourse.bass_utils import run_bass_kernel_spmd
from concourse.masks import make_identity

BF16 = ml_dtypes.bfloat16
F8 = ml_dtypes.float8_e4m3
F32 = np.float32

NS, B, S, E, H, HD = 4, 4, 512, 512, 8, 64
SCALE = HD ** -0.5
LN_EPS = 1e-5
P = 128
HG = 2            # head groups == cores per batch
HPC = 2           # head-pairs per core
HC = H // HG      # heads per core = 4
TG = S // HG      # output rows per core per stream = 256
TS = TG // P      # row tiles per core = 2
ET = E // P       # embedding tiles = 4
KT = S // P       # key/seq tiles = 4
FT = 4 * E // P   # ffn hidden tiles = 16
N_CORES = B * HG

AF = mybir.ActivationFunctionType
ALU = mybir.AluOpType
AX = mybir.AxisListType
DT_BF = mybir.dt.bfloat16
DT_F32 = mybir.dt.float32
DT_F8 = mybir.dt.float8e4
DR = mybir.MatmulPerfMode.DoubleRow


def _build_program(reps=1, ln_triv=True, bf1_zero=True, bf2_zero=True):
    nc = bacc.Bacc("TRN2", target_bir_lowering=False, debug=False)

    def din(name, shape, dt=DT_BF):
        return nc.dram_tensor(name, list(shape), dt, kind="ExternalInput").ap()

    xT_d = din("xT", (NS, P, ET, S), DT_F8)     # xT[n,p,et,s] = x[n,b,s,et*128+p]
    x32_d = din("x32", (NS, P, TS, E), DT_F32)  # x rows t-slice (+bo)
    wq_d = din("wq", (NS, P, ET, HC * HD), DT_F8)  # Wq[n, e, hg*256+c] permuted
    wk_d = din("wk", (NS, P, ET, HC * HD), DT_F8)
    wv_d = din("wv", (NS, P, ET, HC * HD), DT_F8)
    wo_d = din("wo", (NS, P, ET, E))            # Wo[n]/NS, rows e
    w1_d = din("w1", (NS, P, ET, 4 * E))
    w2_d = din("w2", (NS, P, FT, E))
    cmat_d = din("cmat", (P, NS * NS), DT_F32)  # SCALE*inter broadcast on p
    if not ln_triv:
        g1_d = din("g1", (NS, E), DT_F32)
        b1_d = din("b1", (NS, E), DT_F32)
        g2_d = din("g2", (NS, E), DT_F32)
        b2_d = din("b2", (NS, E), DT_F32)
    if not bf1_zero:
        bf1_d = din("bf1", (NS, 4 * E))         # bf1 row (K=1 matmul operand)
    if not bf2_zero:
        bf2_d = din("bf2", (NS, E), DT_F32)
    out_d = nc.dram_tensor("out", [NS, P, TS, E], DT_F32, kind="ExternalOutput").ap()

    with tile.TileContext(nc) as tc:
        with tc.tile_pool(name="const", bufs=1) as const:
            identf = const.tile([P, P], DT_F32)
            make_identity(nc, identf[:])
            cmat_sb = const.tile([P, NS * NS], DT_F32)
            nc.sync.dma_start(cmat_sb[:], cmat_d[:])
            eps_sb = const.tile([P, 1], DT_F32)
            nc.gpsimd.memset(eps_sb[:], LN_EPS)
            # long-lived activations
            r1 = const.tile([P, NS, TS, E], DT_F32)
            r1T = const.tile([P, NS, ET, TG], DT_BF)

            if not ln_triv:
                gbp = tc.alloc_tile_pool(name="gbp", bufs=1)
                g1b = gbp.tile([P, NS, E], DT_F32)
                b1b = gbp.tile([P, NS, E], DT_F32)
                g2b = gbp.tile([P, NS, E], DT_F32)
                b2b = gbp.tile([P, NS, E], DT_F32)
            if not bf2_zero:
                bf2p = tc.alloc_tile_pool(name="bf2p", bufs=1)
                bf2b = bf2p.tile([P, NS, E], DT_F32)
            if not bf1_zero:
                bf1p = tc.alloc_tile_pool(name="bf1p", bufs=1)
                bf1r = bf1p.tile([1, NS, 4 * E], DT_BF)
                ones_row = bf1p.tile([1, TG], DT_BF)
                nc.gpsimd.memset(ones_row[:], 1.0)

            import contextlib
            _loop = tc.For_i(0, reps, 1) if reps > 1 else contextlib.nullcontext()
            with _loop:
              # ---- long-lived per-iteration pools -------------------------
              core = tc.alloc_tile_pool(name="core", bufs=1)
              x32 = core.tile([P, NS, TS, E], DT_F32)
              # fp8 attention operands. qT2[h*32+dl, n, half, s] = q_h[half*32+dl, s]
              qT2 = core.tile([P, NS, 2, S], DT_F8)
              kT2 = core.tile([P, NS, 2, S], DT_F8)
              # engine APs can only start at partition 0/32/64, so head 3's
              # rows (96:128) are DMA-relocated to partitions 0:32 here.
              qT3 = core.tile([P, NS, 2, S], DT_F8)
              kT3 = core.tile([P, NS, 2, S], DT_F8)
              # vex2[k, n, ktpair, h, kthalf, d(+1 ones col)]
              vex2 = core.tile([P, NS, 2, HC, 2, HD + 1], DT_F8)
              wos = core.tile([P, NS, ET, E], DT_BF)
              hp_p = tc.alloc_tile_pool(name="hp_p", bufs=2)  # ffn hidden
              att_p = tc.alloc_tile_pool(name="att_p", bufs=2)
              ex_p = tc.alloc_tile_pool(name="ex_p", bufs=3)
              w1_p = tc.alloc_tile_pool(name="w1_p", bufs=2)   # [P, ET, 1024] halves
              sm = tc.alloc_tile_pool(name="sm", bufs=8)       # [P,*,1] scalars
              tmp = tc.alloc_tile_pool(name="tmp", bufs=2)     # [P,E] f32 temps
              osb = tc.alloc_tile_pool(name="osb", bufs=2)     # out staging

              nc.gpsimd.memset(vex2[:, :, :, :, :, HD:HD + 1], 1.0)

              # ---- psum pools --------------------------------------------
              sps_p = tc.alloc_tile_pool(name="sps_p", bufs=2, space="PSUM")
              vps_p = tc.alloc_tile_pool(name="vps_p", bufs=2, space="PSUM")
              mlp_p = tc.alloc_tile_pool(name="mlp_p", bufs=1, space="PSUM")
              mlp = mlp_p.tile([P, 2, E], DT_F32)           # 2 banks: qkv/wo/ffn

              # ---- input DMAs in priority order --------------------------
              p1 = tc.alloc_tile_pool(name="p1", bufs=1)
              xTs = p1.tile([P, NS, ET, S], DT_F8)
              wqs = p1.tile([P, NS, ET, HC * HD], DT_F8)
              wks = p1.tile([P, NS, ET, HC * HD], DT_F8)
              wvs = p1.tile([P, NS, ET, HC * HD], DT_F8)
              for n in range(NS):
                  nc.sync.dma_start(xTs[:, n], xT_d[n])
                  nc.sync.dma_start(wqs[:, n], wq_d[n])
                  nc.sync.dma_start(wks[:, n], wk_d[n])
                  nc.sync.dma_start(wvs[:, n], wv_d[n])
              for n in range(NS):
                  nc.sync.dma_start(wos[:, n], wo_d[n])
                  nc.sync.dma_start(x32[:, n], x32_d[n])
              w1t = {}  # (n, half) -> tile
              HW1 = 2 * E  # half of 4E
              def fetch_w1(n, half):
                  t = w1_p.tile([P, ET, HW1], DT_BF, tag="w1")
                  nc.sync.dma_start(t[:], w1_d[n][:, :, half * HW1:(half + 1) * HW1])
                  w1t[(n, half)] = t
              w2t = {}
              def fetch_w2(n, half):
                  t = w1_p.tile([P, FT // 2, E], DT_BF, tag="w2")
                  nc.sync.dma_start(t[:], w2_d[n][:, half * (FT // 2):(half + 1) * (FT // 2)])
                  w2t[(n, half)] = t
              fetch_w1(0, 0)
              fetch_w1(0, 1)
              fetch_w2(0, 0)
              fetch_w2(0, 1)
              if not bf1_zero:
                  nc.sync.dma_start(bf1r[:], bf1_d[None, :, :])
              if not ln_triv:
                  for n in range(NS):
                      nc.sync.dma_start(g1b[:, n], g1_d[n].partition_broadcast(P))
                      nc.sync.dma_start(b1b[:, n], b1_d[n].partition_broadcast(P))
                      nc.sync.dma_start(g2b[:, n], g2_d[n].partition_broadcast(P))
                      nc.sync.dma_start(b2b[:, n], b2_d[n].partition_broadcast(P))
              if not bf2_zero:
                  for n in range(NS):
                      nc.sync.dma_start(bf2b[:, n], bf2_d[n].partition_broadcast(P))

              # ---- building blocks ---------------------------------------
              def emit_qkv_qk(n, which):
                  # wq/wk are host-permuted so matmul group `half` yields all
                  # four heads' d-rows [half*32, half*32+32) on partitions
                  # h*32+dl — the DoubleRow layout for fp8 scores.
                  ws = wqs if which == "q" else wks
                  dst = qT2 if which == "q" else kT2
                  for half in range(2):
                      for ep in range(ET // 2):
                          nc.tensor.matmul(
                              mlp[:, half, :],
                              ws[:, n, 2 * ep:2 * ep + 2, half * P:(half + 1) * P],
                              xTs[:, n, 2 * ep:2 * ep + 2, :],
                              start=(ep == 0), stop=(ep == ET // 2 - 1),
                              perf_mode=DR)
                  for half in range(2):
                      nc.scalar.copy(dst[:, n, half], mlp[:, half, :])
                  dst3 = qT3 if which == "q" else kT3
                  nc.scalar.dma_start(dst3[0:32, n], dst[96:128, n])

              def emit_qkv_v(n):
                  for kt in range(KT):
                      dstp = mlp[:, kt // 2, (kt % 2) * 256:(kt % 2) * 256 + 256]
                      for ep in range(ET // 2):
                          nc.tensor.matmul(
                              dstp,
                              xTs[:, n, 2 * ep:2 * ep + 2, kt * P:(kt + 1) * P],
                              wvs[:, n, 2 * ep:2 * ep + 2, :],
                              start=(ep == 0), stop=(ep == ET // 2 - 1),
                              perf_mode=DR)
                  for kt in range(KT):
                      srcp = mlp[:, kt // 2, (kt % 2) * 256:(kt % 2) * 256 + 256]
                      nc.vector.tensor_copy(
                          vex2[:, n, kt // 2, :, kt % 2, 0:HD],
                          srcp.rearrange("p (h d) -> p h d", d=HD))

              # pending attention head: (i, j, h, ex, att_t); its attn@V +
              # normalize are emitted one head-slot later so the PE never
              # hits a head-of-line wait on the exp ACTIVATE.
              pend = [None]

              def _v_steps(pv):
                  # The pending head's attn@V matmuls. Deliberately NOT
                  # DoubleRow: DR disables the fast-weight-load path and at
                  # free-dim 65 the ldweights dominates — plain fp8 with a
                  # full 128-column stationary (FWL) is ~3x faster per MM.
                  i, j, h, exs, att_t, vt = pv
                  for qt in range(KT):
                      for kt in range(KT):
                          nc.tensor.matmul(
                              vt[:, qt, 0:HD + 1],
                              exs[kt // 2][:, kt % 2, qt * P:(qt + 1) * P],
                              vex2[:, j, kt // 2, h, kt % 2, :],
                              start=(kt == 0), stop=(kt == KT - 1))
                          yield

              def _v_finish(pv):
                  i, j, h, exs, att_t, vt = pv
                  r4 = sm.tile([P, KT, 1], DT_F32, tag="r4", bufs=3)
                  nc.vector.reciprocal(r4[:], vt[:, :, HD:HD + 1])
                  for qt in range(KT):
                      dst = att_t[:, qt, h * HD:(h + 1) * HD]
                      if j == 0:
                          nc.vector.tensor_scalar_mul(dst, vt[:, qt, 0:HD],
                                                      r4[:, qt])
                      else:
                          nc.vector.scalar_tensor_tensor(
                              out=dst, in0=vt[:, qt, 0:HD], scalar=r4[:, qt],
                              in1=dst, op0=ALU.mult, op1=ALU.add)

              def flush_pend():
                  if pend[0] is None:
                      return
                  pv = pend[0]
                  pend[0] = None
                  for _ in _v_steps(pv):
                      pass
                  _v_finish(pv)

              def emit_head(i, j, h, att_t):
                  if h == 3:
                      pr = slice(0, 32)
                      kk, qq = kT3, qT3
                  else:
                      pr = slice(h * 32, (h + 1) * 32)
                      kk, qq = kT2, qT2
                  c_ap = cmat_sb[:, (i * NS + j):(i * NS + j + 1)]
                  flush_pend()
                  exs = []
                  for half in range(2):
                      sps = sps_p.tile([P, 2, S], DT_F32, tag="sps", name="sps")
                      for k2 in range(2):
                          kt = half * 2 + k2
                          nc.tensor.matmul(
                              sps[:, k2, :], kk[pr, j, :, kt * P:(kt + 1) * P],
                              qq[pr, i, :, :], start=True, stop=True,
                              perf_mode=DR)
                      ex = ex_p.tile([P, 2, S], DT_F8, tag="ex")
                      nc.scalar.activation(
                          ex[:].rearrange("p a b -> p (a b)"),
                          sps[:].rearrange("p a b -> p (a b)"), AF.Exp, scale=c_ap)
                      exs.append(ex)
                  vt = vps_p.tile([P, KT, P], DT_F32, tag="vps", name="vt")
                  pend[0] = (i, j, h, exs, att_t, vt)

              def ln_stats(src_ap, res_ap, var_sl):
                  # y = src+res; xc = y - mean; var_sl[P,1] = var(y)+eps.
                  # Returns xc. No ACT involvement (avoids table loads).
                  y = tmp.tile([P, E], DT_F32, tag="y")
                  msum = sm.tile([P, 1], DT_F32, tag="msum")
                  nc.vector.scalar_tensor_tensor(
                      out=y[:], in0=src_ap, scalar=1.0, in1=res_ap,
                      op0=ALU.mult, op1=ALU.add, accum_out=msum[:])
                  nm = sm.tile([P, 1], DT_F32, tag="nm")
                  nc.vector.tensor_scalar_mul(nm[:], msum[:], -1.0 / E)
                  xc = tmp.tile([P, E], DT_F32, tag="xc")
                  nc.vector.tensor_scalar_add(xc[:], y[:], nm[:])
                  ssum = sm.tile([P, 1], DT_F32, tag="ssum")
                  nc.vector.scalar_tensor_tensor(
                      out=y[:], in0=xc[:], scalar=1.0, in1=xc[:],
                      op0=ALU.mult, op1=ALU.mult, accum_out=ssum[:])
                  nc.vector.tensor_scalar(
                      out=var_sl, in0=ssum[:], scalar1=1.0 / E, scalar2=LN_EPS,
                      op0=ALU.mult, op1=ALU.add)
                  return xc

              def newton_rsqrt(v, w=TS):
                  # v: [P, w] variance+eps (≈1 for LN'd residual streams).
                  # DVE-only invstd: seed 1.5-0.5v, 4 Newton steps.
                  s = sm.tile([P, w], DT_F32, tag="nr_s", name="nr_s")
                  t = sm.tile([P, w], DT_F32, tag="nr_t", name="nr_t")
                  nc.vector.tensor_scalar(
                      out=s[:], in0=v, scalar1=-0.5, scalar2=1.5,
                      op0=ALU.mult, op1=ALU.add)
                  for _ in range(3):
                      nc.vector.tensor_tensor(t[:], s[:], s[:], ALU.mult)
                      nc.vector.tensor_tensor(t[:], t[:], v, ALU.mult)
                      nc.vector.tensor_scalar(
                          out=t[:], in0=t[:], scalar1=-0.5, scalar2=1.5,
                          op0=ALU.mult, op1=ALU.add)
                      nc.vector.tensor_tensor(s[:], s[:], t[:], ALU.mult)
                  return s

              def ln_out(xc, inv_sl, out_ap, n, which):
                  if ln_triv:
                      nc.vector.tensor_scalar_mul(out_ap, xc[:], inv_sl)
                  else:
                      g = g1b if which == 1 else g2b
                      b = b1b if which == 1 else b2b
                      nc.vector.scalar_tensor_tensor(
                          out=out_ap, in0=xc[:], scalar=inv_sl, in1=g[:, n],
                          op0=ALU.mult, op1=ALU.mult)
                      nc.vector.tensor_add(out_ap, out_ap, b[:, n])

              def make_fillers(i, att_t, drain=False):
                  # Closures for stream i's post-attention work. `head` runs
                  # during stream i+1's pairs; `tail` (gelu+FFN2+LN2, which
                  # gate on all of FFN1) is deferred to the following step so
                  # the gelu never bubbles the ACT queue mid-step. In drain
                  # mode (last stream, nothing left to overlap) the gelu is
                  # split in half and woven between FFN1 chunks, and FFN2
                  # accumulates per gelu-half, to keep the PE warm.
                  hpre = hp_p.tile([P, FT, TG], DT_BF, tag="hpre",
                                   name=f"hpre{i}")
                  hTall = hpre

                  var1 = sm.tile([P, TS], DT_F32, tag="var", name=f"var1_{i}")
                  xcs = {}

                  def wo_ts(ts):
                      def f():
                          wo_ps = mlp[:, ts, :]
                          for qt in range(KT):
                              nc.tensor.matmul(
                                  wo_ps, att_t[:, qt, ts * P:(ts + 1) * P],
                                  wos[:, i, qt], start=(qt == 0),
                                  stop=(qt == KT - 1))
                          xcs[ts] = ln_stats(wo_ps, x32[:, i, ts],
                                             var1[:, ts:ts + 1])
                          if ts == TS - 1:
                              inv = newton_rsqrt(var1[:])
                              for t2 in range(TS):
                                  ln_out(xcs[t2], inv[:, t2:t2 + 1],
                                         r1[:, i, t2], i, 1)
                      return f

                  def tr_ts(ts):
                      def f():
                          for et in range(ET):
                              rt = mlp[:, ts, et * P:(et + 1) * P]
                              nc.tensor.matmul(rt, r1[:, i, ts, et * P:(et + 1) * P],
                                               identf[:], is_transpose=True,
                                               start=True, stop=True)
                          for et in range(ET):
                              rt = mlp[:, ts, et * P:(et + 1) * P]
                              nc.vector.tensor_copy(
                                  r1T[:, i, et, ts * P:(ts + 1) * P], rt)
                      return f

                  def f1_c(c):
                      def f():
                          half, w1s = c // 4, w1t[(i, c // 4)]
                          hp2 = mlp[:, c % 2, :]
                          for s2 in range(2):
                              fs_l = (c % 4) * 2 + s2   # column within half
                              dstp = hp2[:, s2 * 256:(s2 + 1) * 256]
                              for et in range(ET):
                                  nc.tensor.matmul(
                                      dstp, w1s[:, et, fs_l * P:(fs_l + 1) * P],
                                      r1T[:, i, et], start=(et == 0),
                                      stop=(et == ET - 1) if bf1_zero else False)
                              if not bf1_zero:
                                  fs = half * 8 + fs_l
                                  nc.tensor.matmul(
                                      dstp, bf1r[0:1, i, fs * P:(fs + 1) * P],
                                      ones_row[:], start=False, stop=True)
                          hdst = hpre[:, 2 * c:2 * c + 2, :].rearrange(
                              "p a b -> p (a b)")
                          if c % 2 == 0:
                              nc.scalar.copy(hdst, hp2)
                          else:
                              nc.vector.tensor_copy(hdst, hp2)
                          if c == 3 and i < NS - 1:
                              fetch_w1(i + 1, 0)
                          if c == 7 and i < NS - 1:
                              fetch_w1(i + 1, 1)
                          if drain and c in (3, 7):
                              gelu_part(c // 4)
                      return f

                  def gelu_part(g):
                      sl = slice(g * (FT // 2), (g + 1) * (FT // 2))
                      nc.scalar.activation(
                          hTall[:, sl, :].rearrange("p a b -> p (a b)"),
                          hpre[:, sl, :].rearrange("p a b -> p (a b)"),
                          AF.Gelu)

                  def gelu():
                      nc.scalar.activation(
                          hTall[:].rearrange("p a b -> p (a b)"),
                          hpre[:].rearrange("p a b -> p (a b)"), AF.Gelu)

                  out_sb = osb.tile([P, TS, E], DT_F32, tag="osb")
                  var2 = sm.tile([P, TS], DT_F32, tag="var", name=f"var2_{i}")
                  xc2s = {}

                  def f2_ts(ts, fh=None):
                      def f():
                          f2 = mlp[:, ts, :]
                          fts = range(FT) if fh is None else \
                              range(fh * (FT // 2), (fh + 1) * (FT // 2))
                          for ft in fts:
                              w2s = w2t[(i, ft // 8)]
                              nc.tensor.matmul(
                                  f2, hTall[:, ft, ts * P:(ts + 1) * P],
                                  w2s[:, ft % 8], start=(ft == 0),
                                  stop=(ft == FT - 1))
                          if fh == 0:
                              return
                          if bf2_zero:
                              res = r1[:, i, ts]
                          else:
                              res = tmp.tile([P, E], DT_F32, tag="res")
                              nc.vector.tensor_add(res[:], r1[:, i, ts], bf2b[:, i])
                              res = res[:]
                          xc2s[ts] = ln_stats(f2, res, var2[:, ts:ts + 1])
                          if ts == 0 and i < NS - 1:
                              fetch_w2(i + 1, 0)
                          if i == NS - 1:
                              # drain stream: finish each ts independently so
                              # LN2/out of ts0 overlaps FFN2 of ts1
                              inv = newton_rsqrt(var2[:, ts:ts + 1], 1)
                              ln_out(xc2s[ts], inv[:, 0:1], out_sb[:, ts], i, 2)
                              nc.sync.dma_start(out_d[i][:, ts], out_sb[:, ts])
                          elif ts == TS - 1:
                              inv = newton_rsqrt(var2[:])
                              for t2 in range(TS):
                                  ln_out(xc2s[t2], inv[:, t2:t2 + 1],
                                         out_sb[:, t2], i, 2)
                              nc.sync.dma_start(out_d[i], out_sb[:])
                              fetch_w2(i + 1, 1)
                      return f

                  head = [wo_ts(0), wo_ts(1), tr_ts(0), tr_ts(1)]
                  head += [f1_c(c) for c in range(8)]
                  if drain:
                      tail = [f2_ts(0, 0), f2_ts(0, 1), f2_ts(1, 0),
                              f2_ts(1, 1)]
                  else:
                      tail = [gelu, f2_ts(0), f2_ts(1)]
                  return head, tail

              def interleave(tail_prev, head):
                  # gelu(i-1) trails the first exps of the step in the ACT
                  # queue (no step-boundary ACT stall); FFN2(i-1) lands
                  # several slots later so the PE never waits on the gelu.
                  if not tail_prev:
                      return head
                  gl, f2a, f2b = tail_prev
                  return (head[0:2] + [gl] + head[2:6] + [f2a]
                          + head[6:8] + [f2b] + head[8:])

              # ---- emission ----------------------------------------------
              att_tiles = {}

              def get_att(i):
                  if i not in att_tiles:
                      att_tiles[i] = att_p.tile([P, KT, HC * HD], DT_BF,
                                                tag="att", name=f"att{i}")
                  return att_tiles[i]

              def emit_pair(i, j, fillers):
                  att_t = get_att(i)
                  for h in range(HC):
                      if fillers:
                          fillers.pop(0)()
                      emit_head(i, j, h, att_t)

              # prologue: QKV(0), then stream 0's pairs fed by QKV(1..3)
              emit_qkv_qk(0, "q")
              emit_qkv_qk(0, "k")
              emit_qkv_v(0)
              pro = []
              for n in range(1, NS):
                  pro.append(lambda n=n: emit_qkv_qk(n, "q"))
                  pro.append(lambda n=n: emit_qkv_qk(n, "k"))
                  pro.append(lambda n=n: emit_qkv_v(n))
              for j in range(NS):
                  emit_pair(0, j, pro)
              while pro:
                  pro.pop(0)()
              p1.release()

              # steady state: pairs(i+1) interleaved with head-fillers(i) and
              # the deferred tail (gelu/FFN2/LN2) of stream i-1.
              prev_tail = []
              for i in range(NS):
                  flush_pend()  # complete att(i) before its Wo consumes it
                  head, tail = make_fillers(i, get_att(i), drain=(i == NS - 1))
                  fl = interleave(prev_tail, head)
                  prev_tail = tail
                  if i + 1 < NS:
                      for j in range(NS):
                          emit_pair(i + 1, j, fl)
                  for f in fl:
                      f()
              for f in prev_tail:
                  f()

              for p in (mlp_p, vps_p, sps_p, osb, tmp, sm, w1_p, ex_p,
                        att_p, hp_p, core):
                  p.release()

    nc.compile()
    return nc


_NC_CACHE = {}


def _get_nc(reps=1, variant=(True, True, True)):
    key = (reps,) + tuple(variant)
    if key not in _NC_CACHE:
        _NC_CACHE[key] = _build_program(reps, *variant)
    return _NC_CACHE[key]


def _variant(inputs):
    ln_triv = (np.all(np.asarray(inputs["ln1_g"]) == 1.0)
               and np.all(np.asarray(inputs["ln2_g"]) == 1.0)
               and not np.any(np.asarray(inputs["ln1_b"]))
               and not np.any(np.asarray(inputs["ln2_b"])))
    return (ln_triv, not np.any(np.asarray(inputs["bf1"])),
            not np.any(np.asarray(inputs["bf2"])))


def _pack_inputs(x0, x1, x2, x3, Wq, Wk, Wv, Wo, bo, ln1_g, ln1_b, ln2_g, ln2_b,
                 W1, bf1, W2, bf2, inter):
    variant = _variant(dict(ln1_g=ln1_g, ln1_b=ln1_b, ln2_g=ln2_g, ln2_b=ln2_b,
                            bf1=bf1, bf2=bf2))
    ln_triv, bf1_zero, bf2_zero = variant
    x = np.stack([np.asarray(x0), np.asarray(x1), np.asarray(x2),
                  np.asarray(x3)]).astype(F32)  # [NS,B,S,E]
    Wq, Wk, Wv, Wo = (np.asarray(a, F32) for a in (Wq, Wk, Wv, Wo))
    inputs_bo = np.asarray(bo, F32)
    W1, W2 = np.asarray(W1, F32), np.asarray(W2, F32)
    inter = np.asarray(inter, F32)

    def tile_rows(a, nt):
        # [NS, R, C] -> [NS, P, nt, C]
        return np.ascontiguousarray(
            a.reshape(NS, nt, P, a.shape[-1]).transpose(0, 2, 1, 3))

    shared = {
        "wo": tile_rows(Wo / NS, ET).astype(BF16),
        "w1": tile_rows(W1, ET).astype(BF16),
        "w2": tile_rows(W2, FT).astype(BF16),
        "cmat": np.ascontiguousarray(
            np.broadcast_to((inter * SCALE).reshape(1, NS * NS), (P, NS * NS))
        ).astype(F32),
    }
    if not ln_triv:
        shared.update({
            "g1": np.ascontiguousarray(ln1_g, dtype=F32),
            "b1": np.ascontiguousarray(ln1_b, dtype=F32),
            "g2": np.ascontiguousarray(ln2_g, dtype=F32),
            "b2": np.ascontiguousarray(ln2_b, dtype=F32),
        })
    if not bf1_zero:
        shared["bf1"] = np.ascontiguousarray(np.asarray(bf1, F32)).astype(BF16)
    if not bf2_zero:
        shared["bf2"] = np.ascontiguousarray(bf2, dtype=F32)
    # q/k column permutation: matmul group `half` (128 cols) holds all four
    # heads' d-rows [half*32, half*32+32) so the fp8 DoubleRow scores layout
    # qT2[h*32+dl, half, s] falls out of a plain psum->sbuf copy.
    perm = np.array([h * HD + half * 32 + dl
                     for half in range(2) for h in range(HC)
                     for dl in range(32)])
    per_hg = []
    for hg in range(HG):
        cols = slice(hg * HC * HD, (hg + 1) * HC * HD)
        per_hg.append({
            "wq": tile_rows(Wq[:, :, cols][:, :, perm], ET).astype(F8),
            "wk": tile_rows(Wk[:, :, cols][:, :, perm], ET).astype(F8),
            "wv": tile_rows(Wv[:, :, cols], ET).astype(F8),
        })
    xT_b = {}
    for b in range(B):
        xT_b[b] = np.ascontiguousarray(
            x[:, b].transpose(0, 2, 1).reshape(NS, ET, P, S).transpose(0, 2, 1, 3)
        ).astype(F8)
    in_maps = []
    for core in range(N_CORES):
        b, hg = core // HG, core % HG
        xb = x[:, b]  # [NS, S, E]
        x32 = np.ascontiguousarray(
            (xb[:, hg * TG:(hg + 1) * TG] + inputs_bo[:, None, :])
            .reshape(NS, TS, P, E).transpose(0, 2, 1, 3).astype(F32))
        m = {"xT": xT_b[b], "x32": x32}
        m.update(shared)
        m.update(per_hg[hg])
        in_maps.append(m)
    return variant, in_maps


def _unpack_outputs(results):
    full = np.empty((NS, B, S, E), dtype=F32)
    for core in range(N_CORES):
        b, hg = core // HG, core % HG
        o = results[core]["out"]  # [NS, P, TS, E]
        full[:, b, hg * TG:(hg + 1) * TG] = (
            o.transpose(0, 2, 1, 3).reshape(NS, TG, E))
    return tuple(full[n] for n in range(NS))


def _get_exec(variant):
    """Cached jitted shard_map executable for the kernel (mirrors the
    run_bass_kernel_spmd -> run_bass_via_pjrt lowering, but reusable across
    calls instead of re-tracing/jitting per invocation)."""
    key = ("exec",) + tuple(variant)
    if key in _NC_CACHE:
        return _NC_CACHE[key]
    import jax
    from jax.sharding import Mesh, PartitionSpec
    from jax.experimental.shard_map import shard_map
    from concourse import bass2jax
    from concourse import mybir as mb

    nc = _get_nc(1, variant)
    bass2jax.install_neuronx_cc_hook()
    part_name = nc.partition_id_tensor.name if nc.partition_id_tensor else None
    in_names, out_names, out_avals, zero_outs = [], [], [], []
    for alloc in nc.m.functions[0].allocations:
        if not isinstance(alloc, mb.MemoryLocationSet):
            continue
        name = alloc.memorylocations[0].name
        if alloc.kind == "ExternalInput":
            if name != part_name:
                in_names.append(name)
        elif alloc.kind == "ExternalOutput":
            out_names.append(name)
            shape = tuple(alloc.tensor_shape)
            dtype = mb.dt.np(alloc.dtype)
            out_avals.append(jax.core.ShapedArray(shape, dtype))
            zero_outs.append(np.zeros(shape, dtype))
    n_params = len(in_names)
    all_names = in_names + out_names
    if part_name is not None:
        all_names = all_names + [part_name]

    def _body(*args):
        operands = list(args)
        if part_name is not None:
            operands.append(bass2jax.partition_id_tensor())
        outs = bass2jax._bass_exec_p.bind(
            *operands, out_avals=tuple(out_avals), in_names=tuple(all_names),
            out_names=tuple(out_names), lowering_input_output_aliases=(),
            sim_require_finite=True, sim_require_nnan=True, nc=nc)
        return tuple(outs)

    devices = jax.devices()[:N_CORES]
    mesh = Mesh(np.asarray(devices), ("core",))
    spec = PartitionSpec("core")
    donate = tuple(range(n_params, n_params + len(out_names)))
    fn = jax.jit(shard_map(
        _body, mesh=mesh, in_specs=(spec,) * (n_params + len(out_names)),
        out_specs=(spec,) * len(out_names), check_rep=False),
        donate_argnums=donate, keep_unused=True)
    _NC_CACHE[key] = (fn, in_names, out_avals, zero_outs)
    return _NC_CACHE[key]


def kernel(**inputs):
    variant, in_maps = _pack_inputs(**inputs)
    if int(os.environ.get("KERNEL_TRACE", "0")):
        nc = _get_nc(1, variant)
        res = run_bass_kernel_spmd(
            nc, in_maps, core_ids=list(range(N_CORES)), trace=True)
        _NC_CACHE["last_result"] = res
        return _unpack_outputs(res.results)
    fn, in_names, out_avals, zero_outs = _get_exec(variant)
    concat = [np.concatenate([in_maps[c][nm] for c in range(N_CORES)], axis=0)
              for nm in in_names]
    concat += [np.zeros((N_CORES * z.shape[0], *z.shape[1:]), z.dtype)
               for z in zero_outs]
    outs = fn(*concat)
    o = np.asarray(outs[0]).reshape(N_CORES, *out_avals[0].shape)
    return _unpack_outputs([{"out": o[c]} for c in range(N_CORES)])


def bench(inputs, iters=20, reps=1, phases="all"):
    """Time the on-device execution with device-resident inputs.

    Mirrors bass2jax.run_bass_via_pjrt's shard_map(_bass_exec) lowering but
    without output-buffer donation, so the same executable can be re-invoked
    and timed. Returns (min, median) seconds per call. With reps>1 the NEFF
    contains the kernel body replicated; use slopes across reps to cancel
    the fixed axon-RPC dispatch overhead.
    """
    import time
    import jax
    from jax.sharding import Mesh, PartitionSpec, NamedSharding
    from jax.experimental.shard_map import shard_map
    from concourse import bass2jax
    from concourse import mybir as mb

    variant, in_maps = _pack_inputs(**inputs)
    nc = _get_nc(reps, variant)
    bass2jax.install_neuronx_cc_hook()

    part_name = nc.partition_id_tensor.name if nc.partition_id_tensor else None
    in_names, out_names, out_avals, zero_outs = [], [], [], []
    for alloc in nc.m.functions[0].allocations:
        if not isinstance(alloc, mb.MemoryLocationSet):
            continue
        name = alloc.memorylocations[0].name
        if alloc.kind == "ExternalInput":
            if name != part_name:
                in_names.append(name)
        elif alloc.kind == "ExternalOutput":
            out_names.append(name)
            shape = tuple(alloc.tensor_shape)
            dtype = mb.dt.np(alloc.dtype)
            out_avals.append(jax.core.ShapedArray(shape, dtype))
            zero_outs.append(np.zeros(shape, dtype))
    n_params = len(in_names)
    all_names = in_names + out_names
    if part_name is not None:
        all_names = all_names + [part_name]

    def _body(*args):
        operands = list(args)
        if part_name is not None:
            operands.append(bass2jax.partition_id_tensor())
        outs = bass2jax._bass_exec_p.bind(
            *operands, out_avals=tuple(out_avals), in_names=tuple(all_names),
            out_names=tuple(out_names), lowering_input_output_aliases=(),
            sim_require_finite=True, sim_require_nnan=True, nc=nc)
        return tuple(outs)

    devices = jax.devices()[:N_CORES]
    mesh = Mesh(np.asarray(devices), ("core",))
    spec = PartitionSpec("core")
    fn = jax.jit(shard_map(
        _body, mesh=mesh, in_specs=(spec,) * (n_params + len(out_names)),
        out_specs=(spec,) * len(out_names), check_rep=False))
    sh = NamedSharding(mesh, spec)
    concat = [jax.device_put(
        np.concatenate([in_maps[c][nm] for c in range(N_CORES)], axis=0), sh)
        for nm in in_names]
    concat += [jax.device_put(
        np.zeros((N_CORES * z.shape[0], *z.shape[1:]), z.dtype), sh)
        for z in zero_outs]

    out = fn(*concat)  # compile
    jax.block_until_ready(out)
    times = []
    for _ in range(iters):
        t0 = time.perf_counter()
        out = fn(*concat)
        jax.block_until_ready(out)
        times.append(time.perf_counter() - t0)
    times.sort()
    return times[0], times[len(times) // 2]


if __name__ == "__main__":
    import sys
    mode = sys.argv[1] if len(sys.argv) > 1 else "sim"
    sys.path.insert(0, os.path.dirname(os.path.abspath(__file__)))
    import reference

    inputs = {k: np.asarray(v) for k, v in reference.setup_inputs().items()}
    if mode == "sim":
        # Simulate core 0 (b=0, hg=0) with CoreSim and compare to reference.
        # CoreSim has no Gelu; patch exact erf-gelu into its activation visitor.
        import concourse.bass_interp as bass_interp
        from scipy.special import erf as _erf
        _orig_visit = bass_interp.InstructionExecutor.visit_InstActivation

        def _patched(self, instruction, reg_snapshot=None):
            if instruction.func == mybir.ActivationFunctionType.Gelu:
                instruction.func = mybir.ActivationFunctionType.Identity
                try:
                    from concourse.bass_interp import Direction
                    out_ap = instruction.outs[0]
                    res = _orig_visit(self, instruction, reg_snapshot=reg_snapshot)
                    v = self.view_ap(out_ap, Direction.WRITE, instruction,
                                     reg_snapshot=reg_snapshot)
                    x = v[:].astype(np.float32)
                    v[:] = (x * 0.5 * (1.0 + _erf(x / np.sqrt(2.0)))).astype(v.dtype)
                    return res
                finally:
                    instruction.func = mybir.ActivationFunctionType.Gelu
            return _orig_visit(self, instruction, reg_snapshot=reg_snapshot)

        bass_interp.InstructionExecutor.visit_InstActivation = _patched
        from concourse.bass_interp import CoreSim
        core_id = int(sys.argv[2]) if len(sys.argv) > 2 else 0
        variant, in_maps = _pack_inputs(**inputs)
        nc = _get_nc(1, variant)
        sim = CoreSim(nc, trace=False)
        for name, arr in in_maps[core_id].items():
            sim.tensor(name)[:] = arr
        sim.simulate(check_with_hw=False)
        out = sim.tensor("out").copy()
        got = out.transpose(0, 2, 1, 3).reshape(NS, TG, E)
        exp = np.stack([np.asarray(o) for o in reference.reference(**inputs)])
        b, hg = core_id // HG, core_id % HG
        exp_slice = exp[:, b, hg * TG:(hg + 1) * TG]
        err = np.abs(got - exp_slice)
        rel = np.linalg.norm(got - exp_slice) / np.linalg.norm(exp_slice)
        print(f"max abs err: {err.max():.3e}  rel fro err: {rel:.3e}")
    elif mode == "tsim":
        from concourse.timeline_sim import TimelineSim
        variant, in_maps = _pack_inputs(**inputs)
        nc = _get_nc(1, variant)
        ts = TimelineSim(nc, trace=False)
        t = ts.simulate()
        print(f"TimelineSim predicted: {t:.0f} ns")
    else:
        got = kernel(**inputs)
        exp = reference.reference(**inputs)
        for n in range(NS):
            g, e = np.asarray(got[n]), np.asarray(exp[n])
            rel = np.linalg.norm(g - e) / np.linalg.norm(e)
            print(f"out{n}: rel fro err {rel:.3e} max abs {np.abs(g - e).max():.3e}")
